# revision 27
# baseline (speedup 1.0000x reference)
"""LorentzTransformer Trainium2 kernel: 2-way batch DP x 4-way tensor parallel.

Cores 0-3 process batch 0, cores 4-7 batch 1. Within a 4-core group:
attention heads (12 -> 3/rank), d_ff (3072 -> 768/rank) and the tied
vocab logits (50257 -> 12565/rank) are tensor-parallel, with bf16
AllReduce after o_proj and ff_w2.

All matmuls run as float32r (fp32 data at bf16 PE rate). Attention is
computed entirely "k-major": scores are produced transposed (S^T) so no
attention-side transposes are needed; softmax denominators come free by
augmenting V with a ones column; normalization happens on attn^T via a
PE-broadcast of 1/den.
"""

import sys
import numpy as np

sys.path.insert(0, "/opt/trn_rl_repo")

import concourse.bass as bass  # noqa: E402
import concourse.tile as tile  # noqa: E402
from concourse import bacc, mybir  # noqa: E402
from concourse.bass_utils import run_bass_kernel_spmd  # noqa: E402
from concourse.masks import make_identity  # noqa: E402

F32 = mybir.dt.float32
F32R = mybir.dt.float32r
BF16 = mybir.dt.bfloat16
AF = mybir.ActivationFunctionType
ALU = mybir.AluOpType

VOCAB, D, H, NL, L, B = 50257, 768, 12, 4, 1024, 2
DH = D // H
DFF = 4 * D
ALPHA = 0.25
NCORES = 8
TP = 4                      # tensor-parallel width within a batch group
HP = H // TP                # heads per rank (3)
FP = D * HP // H            # attn features per rank (192)
FFP = DFF // TP             # dff per rank (768)
VS = -(-VOCAB // TP)        # vocab per rank (12565; last rank 12562)
VP = -(-VS // 512) * 512    # padded to 512 multiple (12800)
KC = D // 128               # contraction chunks of 128 (6)
TC = L // 128               # token chunks of 128 (8)
EPS = 1e-5
NEG = -1e9

_cached = {}
TRACE = False
LAST_EXEC_NS = None
LAST_TRACE_DIR = None


def _ensure_ntff_hook():
    """The agent image's antenv lacks axon_hooks; synthesize it and
    register the ctypes NTFF profile hook so trace=True works."""
    import types
    if "antenv.axon_hooks" in sys.modules:
        return
    mod = types.ModuleType("antenv.axon_hooks")
    state = {"hook": None}
    mod.set_axon_ntff_profile_hook = lambda h: state.update(hook=h)
    mod.get_axon_ntff_profile_hook = lambda: state["hook"]
    sys.modules["antenv.axon_hooks"] = mod
    try:
        sys.path.insert(0, "/root/.axon_site")
        from trn_agent_boot.trn_boot import _ntff_profile_via_ctypes
        mod.set_axon_ntff_profile_hook(
            _ntff_profile_via_ctypes("/opt/axon/libaxon_pjrt.so"))
    except Exception as e:  # degrade: tracing skipped
        print(f"ntff hook setup failed: {e}")


def _r32(ap):
    return ap.bitcast(F32R)


def _build(flags):
    """Build the SPMD program. flags: dict of bias-gate booleans."""
    nc = bacc.Bacc("TRN2", target_bir_lowering=False, debug=False,
                   num_devices=NCORES)

    # ---- DRAM parameters (per-core values supplied via in_maps) ----
    dx0 = nc.dram_tensor("x0", [L, D], F32, kind="ExternalInput").ap()
    dwqk = nc.dram_tensor("wqk", [NL, KC, 128, 384], F32R, kind="ExternalInput").ap()
    dwv = nc.dram_tensor("wv", [NL, KC, 128, 256], F32R, kind="ExternalInput").ap()
    dwo = nc.dram_tensor("wo", [NL, 2, 128, D], F32R, kind="ExternalInput").ap()
    dw1 = nc.dram_tensor("w1", [NL, KC, 128, FFP], F32R, kind="ExternalInput").ap()
    dw2 = nc.dram_tensor("w2", [NL, KC, 128, D], F32R, kind="ExternalInput").ap()
    demb = nc.dram_tensor("embT", [KC, 128, VP], F32R, kind="ExternalInput").ap()
    dqkvb = nc.dram_tensor("qkvb", [128, NL * 4], F32, kind="ExternalInput").ap()
    df1b = nc.dram_tensor("f1b", [128, NL * 6], F32, kind="ExternalInput").ap()
    dmsk = nc.dram_tensor("mskT", [4, 128, 512], BF16, kind="ExternalInput").ap()
    dexp = nc.dram_tensor("expnd", [96, 192], F32R, kind="ExternalInput").ap()
    dob = dfb2 = dlgb = None
    if flags["ob"]:
        dob = nc.dram_tensor("ob", [NL, 1, D], F32, kind="ExternalInput").ap()
    if flags["fb2"]:
        dfb2 = nc.dram_tensor("fb2", [NL, 1, D], F32, kind="ExternalInput").ap()
    if flags["lgb"]:
        dlgb = nc.dram_tensor("lgb", [1, VP], F32, kind="ExternalInput").ap()
    dlog = nc.dram_tensor("logits", [L, VP], F32, kind="ExternalOutput").ap()

    groups = [[0, 1, 2, 3], [4, 5, 6, 7]]

    from contextlib import ExitStack
    with tile.TileContext(nc) as tc, ExitStack() as es:
        cst = es.enter_context(tc.tile_pool(name="cst", bufs=1))
        ph = es.enter_context(tc.tile_pool(name="ph", bufs=1))
        pxT = es.enter_context(tc.tile_pool(name="pxT", bufs=1))
        px2T = es.enter_context(tc.tile_pool(name="px2T", bufs=1))
        pqk = es.enter_context(tc.tile_pool(name="pqk", bufs=1))
        pv = es.enter_context(tc.tile_pool(name="pv", bufs=1))
        pawT = es.enter_context(tc.tile_pool(name="pawT", bufs=3))
        pattn = es.enter_context(tc.tile_pool(name="pattn", bufs=1))
        psml = es.enter_context(tc.tile_pool(name="psml", bufs=2))
        pscr = es.enter_context(tc.tile_pool(name="pscr", bufs=2))
        pff = es.enter_context(tc.tile_pool(name="pff", bufs=1))
        pcast = es.enter_context(tc.tile_pool(name="pcast", bufs=2))
        pcin = es.enter_context(tc.tile_pool(name="pcin", bufs=2))
        pw = es.enter_context(tc.tile_pool(name="pw", bufs=1))
        pemb = es.enter_context(tc.tile_pool(name="pemb", bufs=6))
        pps = es.enter_context(tc.tile_pool(name="pps", bufs=6, space="PSUM"))
        pav = es.enter_context(tc.tile_pool(name="pav", bufs=2, space="PSUM"))
        pdram = es.enter_context(tc.tile_pool(name="pdram", bufs=2, space="DRAM"))

        dma = nc.sync.dma_start

        # ---- constants ----
        ident = cst.tile([128, 128], F32, tag="ident")
        make_identity(nc, ident[:])
        msk = cst.tile([128, 4 * 512], BF16, tag="msk")
        for j in range(4):
            dma(out=msk[:, j * 512:(j + 1) * 512], in_=dmsk[j])
        epst = cst.tile([128, 1], F32, tag="epst")
        nc.vector.memset(epst[:], EPS)
        ones1 = cst.tile([128, 1], F32, tag="ones1")
        nc.vector.memset(ones1[:], 1.0)
        qkvb = cst.tile([128, NL * 4], F32, tag="qkvb")
        dma(out=qkvb[:], in_=dqkvb[:])
        f1b = cst.tile([128, NL * 6], F32, tag="f1b")
        dma(out=f1b[:], in_=df1b[:])
        ob_sb = fb2_sb = lgb_sb = None
        if flags["ob"]:
            ob_sb = cst.tile([128, NL * D], F32, tag="ob")
            for i in range(NL):
                dma(out=ob_sb[:, i * D:(i + 1) * D],
                    in_=dob[i].to_broadcast([128, D]))
        if flags["fb2"]:
            fb2_sb = cst.tile([128, NL * D], F32, tag="fb2")
            for i in range(NL):
                dma(out=fb2_sb[:, i * D:(i + 1) * D],
                    in_=dfb2[i].to_broadcast([128, D]))
        if flags["lgb"]:
            lgb_sb = cst.tile([128, VP], F32, tag="lgb")
            dma(out=lgb_sb[:], in_=dlgb.to_broadcast([128, VP]))

        # inv-broadcast expander one-hots (constants, host supplied)
        expnd = cst.tile([96, 192], F32R, tag="expnd")
        dma(out=expnd[:], in_=dexp[:])
        exp01 = expnd[:, 0:128]
        exp2 = expnd[:, 128:192]

        # ---- residual stream ----
        h = ph.tile([128, TC * D], F32, tag="h")
        for t in range(TC):
            dma(out=h[:, t * D:(t + 1) * D], in_=dx0[t * 128:(t + 1) * 128, :])

        def layernorm_T(src_tile, xT, xT_off, xT_stride, tcs, gelu_none=None):
            """LN (z only; gamma/beta folded) of token-chunks `tcs` of
            src_tile ([128, len*768] slices) into transposed xT slices
            [:, kc*xT_stride + xT_off + 128*j]."""
            st = psml.tile([128, 40], F32, tag="st")
            n = len(tcs)
            for j, t in enumerate(tcs):
                xs = src_tile[:, t * D:(t + 1) * D]
                nc.vector.tensor_reduce(out=st[:, j:j + 1], in_=xs,
                                        axis=mybir.AxisListType.X, op=ALU.add)
                scr = pscr.tile([128, D], F32, tag="scr")
                nc.vector.scalar_tensor_tensor(
                    out=scr[:], in0=xs, scalar=1.0, in1=xs,
                    op0=ALU.mult, op1=ALU.mult,
                    accum_out=st[:, 8 + j:9 + j])
            mu = st[:, 16:16 + n]
            ex2 = st[:, 24:24 + n]
            nc.scalar.mul(out=mu, in_=st[:, 0:n], mul=1.0 / D)
            nc.scalar.mul(out=ex2, in_=st[:, 8:8 + n], mul=1.0 / D)
            musq = st[:, 32:32 + n]
            nc.scalar.square(out=musq, in_=mu)
            var = st[:, 8:8 + n]          # reuse
            nc.vector.scalar_tensor_tensor(out=var, in0=ex2, scalar=1.0,
                                           in1=musq, op0=ALU.mult,
                                           op1=ALU.subtract)
            sd = st[:, 24:24 + n]         # reuse
            nc.scalar.activation(out=sd, in_=var, func=AF.Sqrt,
                                 bias=epst[:, 0:1])
            rstd = st[:, 0:n]             # reuse
            nc.vector.reciprocal(out=rstd, in_=sd)
            nmr = st[:, 32:32 + n]        # reuse (musq dead)
            nc.vector.scalar_tensor_tensor(out=nmr, in0=mu, scalar=-1.0,
                                           in1=rstd, op0=ALU.mult,
                                           op1=ALU.mult)
            for j, t in enumerate(tcs):
                z = pscr.tile([128, D], F32, tag="scr")
                nc.scalar.activation(out=z[:], in_=src_tile[:, t * D:(t + 1) * D],
                                     func=AF.Identity,
                                     bias=nmr[:, j:j + 1], scale=rstd[:, j:j + 1])
                # transpose 128x768 -> 6 PE transposes, packed 4-per-psum-bank
                for g in range(2):
                    ptr = pps.tile([128, 512], F32, tag="ps")
                    nk = 4 if g == 0 else 2
                    for q in range(nk):
                        kc = g * 4 + q
                        nc.tensor.transpose(ptr[:, q * 128:(q + 1) * 128],
                                            z[:, kc * 128:(kc + 1) * 128],
                                            ident[:])
                    for q in range(nk):
                        kc = g * 4 + q
                        nc.scalar.copy(
                            out=xT[:, kc * xT_stride + xT_off + 128 * j:
                                   kc * xT_stride + xT_off + 128 * (j + 1)],
                            in_=ptr[:, q * 128:(q + 1) * 128])

        # ================= layers =================
        for i in range(NL):
            # --- weights for this layer ---
            wqk = pw.tile([128, KC * 384], F32R, tag="wqk")
            for kc in range(KC):
                dma(out=wqk[:, kc * 384:(kc + 1) * 384], in_=dwqk[i, kc])
            wv = pw.tile([128, KC * 256], F32R, tag="wv")
            for kc in range(KC):
                dma(out=wv[:, kc * 256:(kc + 1) * 256], in_=dwv[i, kc])
            wo = pw.tile([128, 2 * D], F32R, tag="wo")
            for kc in range(2):
                dma(out=wo[:, kc * D:(kc + 1) * D], in_=dwo[i, kc])
            w1 = pw.tile([128, KC * FFP], F32R, tag="w1")
            for kc in range(KC):
                dma(out=w1[:, kc * FFP:(kc + 1) * FFP], in_=dw1[i, kc])
            w2 = pw.tile([128, KC * D], F32R, tag="w2")
            for kc in range(KC):
                dma(out=w2[:, kc * D:(kc + 1) * D], in_=dw2[i, kc])

            # --- phase A: LN1 + transpose -> xT ([128, KC*1024]) ---
            xT = pxT.tile([128, KC * L], F32R, tag="xT")
            layernorm_T(h, xT, 0, L, list(range(TC)))

            # --- phase B: q/k/v projections ---
            # qk tile blocks (by column range m*L): q01 | q2 | k01 | k2
            MOFF = [0, 128, 192, 320]
            MSZ = [128, 64, 128, 64]
            qk = pqk.tile([128, 4 * L], F32R, tag="qk")
            for m in range(4):
                msz = MSZ[m]
                for nh in range(2):
                    pmm = pps.tile([msz, 512], F32, tag="ps")
                    for kc in range(KC):
                        nc.tensor.matmul(
                            pmm[:],
                            _r32(wqk[:, kc * 384 + MOFF[m]:
                                     kc * 384 + MOFF[m] + msz]),
                            _r32(xT[:, kc * L + nh * 512:kc * L + (nh + 1) * 512]),
                            start=(kc == 0), stop=(kc == KC - 1))
                    nc.scalar.activation(
                        out=qk[0:msz, m * L + nh * 512:m * L + (nh + 1) * 512],
                        in_=pmm[:], func=AF.Identity,
                        bias=qkvb[0:msz, i * 4 + m:i * 4 + m + 1])
            # v (token-major) + ones column per head: [128, 8*195]
            v = pv.tile([128, TC * 195], F32R, tag="v")
            for t in range(TC):
                for hh in range(HP):
                    nc.scalar.copy(out=v[:, t * 195 + hh * 65 + 64:
                                         t * 195 + hh * 65 + 65],
                                   in_=ones1[:])
                pmm = pps.tile([128, 256], F32, tag="ps")
                for kc in range(KC):
                    nc.tensor.matmul(
                        pmm[:],
                        _r32(xT[:, kc * L + t * 128:kc * L + (t + 1) * 128]),
                        _r32(wv[:, kc * 256:(kc + 1) * 256]),
                        start=(kc == 0), stop=(kc == KC - 1))
                for hh in range(HP):
                    nc.scalar.copy(
                        out=v[:, t * 195 + hh * 65:t * 195 + hh * 65 + 64],
                        in_=pmm[:, hh * 64:(hh + 1) * 64])

            # head slices: q in blocks 0/1, k in blocks 2/3 (same bases)
            def qsl(hh, c0, c1):
                m, p0 = [(0, 0), (0, 64), (1, 0)][hh]
                return qk[p0:p0 + 64, m * L + c0:m * L + c1]

            def ksl(hh, c0, c1):
                m, p0 = [(2, 0), (2, 64), (3, 0)][hh]
                return qk[p0:p0 + 64, m * L + c0:m * L + c1]

            # --- phase C: attention (k-major) ---
            attnA = pattn.tile([128, L], F32R, tag="attnA")
            attnB = pattn.tile([64, L], F32R, tag="attnB")
            den = psml.tile([96, L], F32, tag="den")
            nc.vector.memset(den[:], 1.0)
            inv = psml.tile([96, L], F32R, tag="inv")
            for hh in range(HP):
                for qc in range(2):
                    nkj = 4 * qc + 4
                    pavt = pav.tile([65, 512], F32, tag="av")
                    for kj in range(nkj):
                        pst = pps.tile([128, 512], F32, tag="ps")
                        nc.tensor.matmul(
                            pst[:],
                            _r32(ksl(hh, kj * 128, (kj + 1) * 128)),
                            _r32(qsl(hh, qc * 512, (qc + 1) * 512)),
                            start=True, stop=True)
                        awT = pawT.tile([128, 512], F32R, tag="awT")
                        off = kj - 4 * qc
                        if off >= 0:  # diagonal-overlap block: add causal mask
                            nc.vector.scalar_tensor_tensor(
                                out=awT[:], in0=pst[:], scalar=1.0,
                                in1=msk[:, off * 512:(off + 1) * 512],
                                op0=ALU.mult, op1=ALU.add)
                            nc.scalar.activation(out=awT[:], in_=awT[:],
                                                 func=AF.Exp)
                        else:
                            nc.scalar.activation(out=awT[:], in_=pst[:],
                                                 func=AF.Exp)
                        nc.tensor.matmul(
                            pavt[:],
                            _r32(v[:, kj * 195 + hh * 65:kj * 195 + (hh + 1) * 65]),
                            _r32(awT[:]),
                            start=(kj == 0), stop=(kj == nkj - 1))
                    # den row -> den[hh, qc*512:...]
                    nc.scalar.copy(out=den[32 * hh:32 * hh + 1, qc * 512:(qc + 1) * 512],
                                   in_=pavt[64:65, :])
                    # stash raw attnT (normalized below)
                    dst = attnA[hh * 64:(hh + 1) * 64, qc * 512:(qc + 1) * 512] \
                        if hh < 2 else attnB[:, qc * 512:(qc + 1) * 512]
                    nc.scalar.copy(out=dst, in_=pavt[0:64, :])
            with nc.allow_low_precision(reason="softmax inv in f32r"):
                nc.vector.reciprocal(out=inv[:], in_=den[:])
            for qc in range(2):
                pib0 = pps.tile([128, 512], F32, tag="ps")
                nc.tensor.matmul(pib0[:], _r32(exp01),
                                 _r32(inv[:, qc * 512:(qc + 1) * 512]),
                                 start=True, stop=True)
                nc.vector.scalar_tensor_tensor(
                    out=attnA[:, qc * 512:(qc + 1) * 512],
                    in0=attnA[:, qc * 512:(qc + 1) * 512], scalar=1.0,
                    in1=pib0[:], op0=ALU.mult, op1=ALU.mult)
                pib1 = pps.tile([64, 512], F32, tag="ps")
                nc.tensor.matmul(pib1[:], _r32(exp2),
                                 _r32(inv[:, qc * 512:(qc + 1) * 512]),
                                 start=True, stop=True)
                nc.vector.scalar_tensor_tensor(
                    out=attnB[:, qc * 512:(qc + 1) * 512],
                    in0=attnB[:, qc * 512:(qc + 1) * 512], scalar=1.0,
                    in1=pib1[:], op0=ALU.mult, op1=ALU.mult)

            # --- phase D: o_proj partials + AllReduce (bf16, 2 halves) ---
            obnc = [pdram.tile([512, D], BF16, tag="obnc",
                               name=f"obnc_{i}_{k}") for k in range(2)]
            ored = [pdram.tile([512, D], BF16, tag="ored",
                               name=f"ored_{i}_{k}") for k in range(2)]
            for t in range(TC):
                oc = pcast.tile([128, D], BF16, tag="oc")
                for nn_ in range(2):
                    n0, n1 = nn_ * 512, min(D, (nn_ + 1) * 512)
                    pmo = pps.tile([128, n1 - n0], F32, tag="ps")
                    nc.tensor.matmul(pmo[:],
                                     _r32(attnA[:, t * 128:(t + 1) * 128]),
                                     _r32(wo[:, n0:n1]),
                                     start=True, stop=False)
                    nc.tensor.matmul(pmo[:],
                                     _r32(attnB[:, t * 128:(t + 1) * 128]),
                                     _r32(wo[0:64, D + n0:D + n1]),
                                     start=False, stop=True)
                    nc.scalar.copy(out=oc[:, n0:n1], in_=pmo[:])
                half, row = t // 4, (t % 4) * 128
                dma(out=obnc[half][row:row + 128, :], in_=oc[:])
            for half in range(2):
                nc.gpsimd.collective_compute(
                    "AllReduce", ALU.add, replica_groups=groups,
                    ins=[obnc[half].opt()], outs=[ored[half].opt()])
            for t in range(TC):
                half, row = t // 4, (t % 4) * 128
                ci = pcin.tile([128, D], BF16, tag="ci")
                dma(out=ci[:], in_=ored[half][row:row + 128, :])
                nc.vector.scalar_tensor_tensor(
                    out=h[:, t * D:(t + 1) * D], in0=h[:, t * D:(t + 1) * D],
                    scalar=1.0, in1=ci[:], op0=ALU.mult, op1=ALU.add)
                if flags["ob"]:
                    nc.vector.scalar_tensor_tensor(
                        out=h[:, t * D:(t + 1) * D],
                        in0=h[:, t * D:(t + 1) * D], scalar=1.0,
                        in1=ob_sb[:, i * D:(i + 1) * D],
                        op0=ALU.mult, op1=ALU.add)

            # --- phase E: LN2 + FFN + AllReduce ---
            fbnc = [pdram.tile([512, D], BF16, tag="fbnc",
                               name=f"fbnc_{i}_{k}") for k in range(2)]
            fred = [pdram.tile([512, D], BF16, tag="fred",
                               name=f"fred_{i}_{k}") for k in range(2)]
            for tp_ in range(4):                      # 256-token chunks
                x2T = px2T.tile([128, KC * 256], F32R, tag="x2T")
                layernorm_T(h, x2T, 0, 256, [2 * tp_, 2 * tp_ + 1])
                ff = pff.tile([128, KC * 256], F32R, tag="ff")
                for mc in range(KC):
                    pmf = pps.tile([128, 256], F32, tag="ps")
                    for kc in range(KC):
                        nc.tensor.matmul(
                            pmf[:],
                            _r32(w1[:, kc * FFP + mc * 128:kc * FFP + (mc + 1) * 128]),
                            _r32(x2T[:, kc * 256:(kc + 1) * 256]),
                            start=(kc == 0), stop=(kc == KC - 1))
                    nc.scalar.activation(out=ff[:, mc * 256:(mc + 1) * 256],
                                         in_=pmf[:], func=AF.Gelu,
                                         bias=f1b[:, i * 6 + mc:i * 6 + mc + 1])
                for sub in range(2):                  # 128-token halves
                    t = 2 * tp_ + sub
                    fc = pcast.tile([128, D], BF16, tag="fc")
                    for nn_ in range(2):
                        n0, n1 = nn_ * 512, min(D, (nn_ + 1) * 512)
                        pmf2 = pps.tile([128, n1 - n0], F32, tag="ps")
                        for kc in range(KC):
                            nc.tensor.matmul(
                                pmf2[:],
                                _r32(ff[:, kc * 256 + sub * 128:kc * 256 + sub * 128 + 128]),
                                _r32(w2[:, kc * D + n0:kc * D + n1]),
                                start=(kc == 0), stop=(kc == KC - 1))
                        nc.scalar.copy(out=fc[:, n0:n1], in_=pmf2[:])
                    half, row = t // 4, (t % 4) * 128
                    dma(out=fbnc[half][row:row + 128, :], in_=fc[:])
            for half in range(2):
                nc.gpsimd.collective_compute(
                    "AllReduce", ALU.add, replica_groups=groups,
                    ins=[fbnc[half].opt()], outs=[fred[half].opt()])
            for t in range(TC):
                half, row = t // 4, (t % 4) * 128
                ci = pcin.tile([128, D], BF16, tag="ci")
                dma(out=ci[:], in_=fred[half][row:row + 128, :])
                nc.vector.scalar_tensor_tensor(
                    out=h[:, t * D:(t + 1) * D], in0=h[:, t * D:(t + 1) * D],
                    scalar=1.0, in1=ci[:], op0=ALU.mult, op1=ALU.add)
                if flags["fb2"]:
                    nc.vector.scalar_tensor_tensor(
                        out=h[:, t * D:(t + 1) * D],
                        in0=h[:, t * D:(t + 1) * D], scalar=1.0,
                        in1=fb2_sb[:, i * D:(i + 1) * D],
                        op0=ALU.mult, op1=ALU.add)

        # ================= final LN + logits =================
        zT = pxT.tile([128, KC * L], F32R, tag="xT")
        layernorm_T(h, zT, 0, L, list(range(TC)))
        nvc = -(-VP // 512)
        for vc in range(nvc):
            v0, v1 = vc * 512, min(VP, (vc + 1) * 512)
            et = [pemb.tile([128, v1 - v0], F32R, tag="emb",
                            name=f"emb_{vc}_{k}") for k in range(KC)]
            for kc in range(KC):
                dma(out=et[kc][:], in_=demb[kc, :, v0:v1])
            for t in range(TC):
                pml = pps.tile([128, v1 - v0], F32, tag="ps")
                for kc in range(KC):
                    nc.tensor.matmul(
                        pml[:],
                        _r32(zT[:, kc * L + t * 128:kc * L + (t + 1) * 128]),
                        _r32(et[kc][:]),
                        start=(kc == 0), stop=(kc == KC - 1))
                lg = pscr.tile([128, D], F32, tag="scr",
                               name=f"lgout_{vc}_{t}")[:, 0:v1 - v0]
                if flags["lgb"]:
                    nc.vector.scalar_tensor_tensor(
                        out=lg[:], in0=pml[:], scalar=1.0,
                        in1=lgb_sb[:, v0:v1], op0=ALU.mult, op1=ALU.add)
                elif t % 2 == 0:
                    nc.vector.tensor_scalar_add(out=lg[:], in0=pml[:],
                                                scalar1=0.0)
                else:
                    nc.scalar.copy(out=lg[:], in_=pml[:])
                dma(out=dlog[t * 128:(t + 1) * 128, v0:v1], in_=lg[:])

    nc.compile()
    return nc


def _prep_inputs(tokens, timelike_mask, embed, pos_emb, wq, wk, wv, wo,
                 ln1_g, ln1_b, ln2_g, ln2_b, ff_w1, ff_b1, ff_w2, ff_b2,
                 lnf_g, lnf_b):
    """Fold scales into weights and build the 8 per-core input maps."""
    f32 = np.float32
    tokens = np.asarray(tokens)
    scale = float(np.sqrt(DH))
    flags = {
        "ob": bool(np.any(ln1_b)),
        "fb2": bool(np.any(ff_b2) or np.any(ln2_b)),
        "lgb": bool(np.any(lnf_b)),
    }

    # x0 per batch
    x0 = (np.asarray(embed)[tokens] + np.asarray(pos_emb)[None, :L]).astype(f32)

    # causal mask tiles (transposed layout): msk[o][i,j] = 0 if j >= i+o*128
    mskT = np.full((4, 128, 512), NEG, f32)
    for o in range(4):
        i_idx = np.arange(128)[:, None]
        j_idx = np.arange(512)[None, :]
        mskT[o][j_idx >= i_idx + o * 128] = 0.0
    import ml_dtypes
    mskT = mskT.astype(ml_dtypes.bfloat16)

    per_rank = []
    for t in range(TP):
        fs, fe = t * FP, (t + 1) * FP          # attn feature slice
        ffs, ffe = t * FFP, (t + 1) * FFP      # dff slice
        vs = t * VS
        ve = min(VOCAB, (t + 1) * VS)

        wqk_r = np.zeros((NL, KC, 128, 384), f32)
        wv_r = np.zeros((NL, KC, 128, 256), f32)
        wo_r = np.zeros((NL, 2, 128, D), f32)
        w1_r = np.zeros((NL, KC, 128, FFP), f32)
        w2_r = np.zeros((NL, KC, 128, D), f32)
        qkvb_r = np.zeros((128, NL * 4), f32)
        f1b_r = np.zeros((128, NL * 6), f32)
        ob_r = np.zeros((NL, 1, D), f32)
        fb2_r = np.zeros((NL, 1, D), f32)

        for i in range(NL):
            s_lor = (1.0 - 2.0 * ALPHA *
                     np.asarray(timelike_mask)[i].astype(f32)) / scale
            wq_g = (np.asarray(wq)[i] * s_lor[:, None]) * \
                np.asarray(ln1_g)[i][None, :]
            wk_g = np.asarray(wk)[i] * np.asarray(ln1_g)[i][None, :]
            wv_g = np.asarray(wv)[i] * np.asarray(ln1_g)[i][None, :]
            qb = wq_g[fs:fe] @ np.asarray(ln1_b)[i]
            kb = wk_g[fs:fe] @ np.asarray(ln1_b)[i]
            # packed qk lhsT [768, 384]: cols q01 | q2 | k01 | k2
            qkT = wq_g[fs:fe].T          # [768, 192]
            kkT = wk_g[fs:fe].T
            pack = np.concatenate([qkT, kkT], 1)
            wqk_r[i] = pack.reshape(KC, 128, 384)
            wv_r[i, :, :, 0:FP] = wv_g[fs:fe].T.reshape(KC, 128, FP)
            woT = np.asarray(wo)[i][:, fs:fe].T   # [192, 768]
            wo_r[i, 0] = woT[0:128]
            wo_r[i, 1, 0:64] = woT[128:192]
            w1_g = np.asarray(ff_w1)[i][ffs:ffe] * np.asarray(ln2_g)[i][None, :]
            w1_r[i] = w1_g.T.reshape(KC, 128, FFP)
            w2_r[i] = np.asarray(ff_w2)[i][:, ffs:ffe].T.reshape(KC, 128, D)
            qkvb_r[:, i * 4 + 0] = qb[0:128]
            qkvb_r[0:64, i * 4 + 1] = qb[128:192]
            qkvb_r[:, i * 4 + 2] = kb[0:128]
            qkvb_r[0:64, i * 4 + 3] = kb[128:192]
            b1 = w1_g @ np.asarray(ln2_b)[i] + np.asarray(ff_b1)[i][ffs:ffe]
            f1b_r[:, i * 6:(i + 1) * 6] = b1.reshape(KC, 128).T
            vb = wv_g[fs:fe] @ np.asarray(ln1_b)[i]
            ob_r[i, 0] = np.asarray(wo)[i][:, fs:fe] @ vb
            fb2_r[i, 0] = np.asarray(ff_b2)[i] / 1.0  # added once per core
            # note: fb2 added on all 4 ranks post-AR -> divide by TP? No:
            # post-AR add happens on each core independently on its own h
            # copy, so full fb2 is correct (h is replicated, not summed).

        embT_r = np.zeros((KC, 128, VP), f32)
        esl = (np.asarray(embed)[vs:ve] * np.asarray(lnf_g)[None, :]).T
        embT_r[:, :, 0:ve - vs] = esl.reshape(KC, 128, ve - vs)
        lgb_r = np.zeros((1, VP), f32)
        lgb_r[0, 0:ve - vs] = np.asarray(embed)[vs:ve] @ np.asarray(lnf_b)

        expnd = np.zeros((96, 192), f32)
        expnd[0, 0:64] = 1.0
        expnd[32, 64:128] = 1.0
        expnd[64, 128:192] = 1.0
        per_rank.append(dict(
            wqk=wqk_r, wv=wv_r, wo=wo_r, w1=w1_r, w2=w2_r,
            qkvb=qkvb_r, f1b=f1b_r, embT=embT_r, expnd=expnd,
            ob=ob_r, fb2=fb2_r, lgb=lgb_r, mskT=mskT))

    in_maps = []
    for c in range(NCORES):
        g, t = c // TP, c % TP
        m = dict(per_rank[t])
        m["x0"] = np.ascontiguousarray(x0[g])
        if not flags["ob"]:
            m.pop("ob")
        if not flags["fb2"]:
            m.pop("fb2")
        if not flags["lgb"]:
            m.pop("lgb")
        in_maps.append(m)
    return in_maps, flags


def kernel(**inputs):
    in_maps, flags = _prep_inputs(**inputs)
    key = tuple(sorted(flags.items()))
    if key not in _cached:
        _cached[key] = _build(flags)
    nc = _cached[key]
    global LAST_EXEC_NS, LAST_TRACE_DIR
    if TRACE:
        _ensure_ntff_hook()
        import tempfile
        tdir = tempfile.mkdtemp(prefix="lorentz_trace_")
        res = run_bass_kernel_spmd(nc, in_maps, core_ids=list(range(NCORES)),
                                   trace=True, tmpdir=tdir)
        LAST_EXEC_NS = res.exec_time_ns
        LAST_TRACE_DIR = tdir
    else:
        res = run_bass_kernel_spmd(nc, in_maps, core_ids=list(range(NCORES)))
    out = np.zeros((B, L, VOCAB), np.float32)
    for c in range(NCORES):
        g, t = c // TP, c % TP
        vs = t * VS
        ve = min(VOCAB, (t + 1) * VS)
        out[g, :, vs:ve] = res.results[c]["logits"][:, 0:ve - vs]
    return out


# revision 34
# speedup vs baseline: 1.1301x; 1.1301x over previous
"""LorentzTransformer Trainium2 kernel: 2-way batch DP x 4-way tensor parallel.

Cores 0-3 process batch 0, cores 4-7 batch 1. Within a 4-core group:
attention heads (12 -> 3/rank), d_ff (3072 -> 768/rank) and the tied
vocab logits (50257 -> 12565/rank) are tensor-parallel, with bf16
AllReduce after o_proj and ff_w2, each split into 4 token-chunks that
pipeline against LN/FF compute. The AR-consuming h-update is fused
with the next LayerNorm's sum reduction (one DVE pass).

Matmul operands are bf16 (fp32 PSUM accumulation); LN statistics and
the residual stream stay fp32. Attention is computed "k-major": scores
are produced transposed (S^T) so no attention-side transposes exist;
softmax denominators come free from a ones-column appended to V; each
(head, q-chunk) normalizes attn^T via a K=1 PE broadcast of 1/den.
Fully-masked causal columns are skipped in the S^T matmuls.
"""

import sys
import numpy as np

sys.path.insert(0, "/opt/trn_rl_repo")

import concourse.bass as bass  # noqa: E402,F401
import concourse.tile as tile  # noqa: E402
from concourse import bacc, mybir  # noqa: E402
from concourse.bass_utils import run_bass_kernel_spmd  # noqa: E402
from concourse.masks import make_identity  # noqa: E402

F32 = mybir.dt.float32
BF16 = mybir.dt.bfloat16
AF = mybir.ActivationFunctionType
ALU = mybir.AluOpType

VOCAB, D, H, NL, L, B = 50257, 768, 12, 4, 1024, 2
DH = D // H
DFF = 4 * D
ALPHA = 0.25
NCORES = 8
TP = 4
HP = H // TP                # heads per rank (3)
FP = D * HP // H            # attn features per rank (192)
FFP = DFF // TP             # dff per rank (768)
VS = -(-VOCAB // TP)        # vocab per rank (12565; last rank 12562)
VP = -(-VS // 512) * 512    # padded to 512 multiple (12800)
KC = D // 128               # contraction chunks (6)
TC = L // 128               # token chunks of 128 (8)
EPS = 1e-5

_cached = {}
TRACE = False
LAST_EXEC_NS = None
LAST_TRACE_DIR = None
_uid = [0]


def _nm(p):
    _uid[0] += 1
    return f"{p}_{_uid[0]}"


def _ensure_ntff_hook():
    import types
    if "antenv.axon_hooks" in sys.modules:
        return
    mod = types.ModuleType("antenv.axon_hooks")
    state = {"hook": None}
    mod.set_axon_ntff_profile_hook = lambda h: state.update(hook=h)
    mod.get_axon_ntff_profile_hook = lambda: state["hook"]
    sys.modules["antenv.axon_hooks"] = mod
    try:
        sys.path.insert(0, "/root/.axon_site")
        from trn_agent_boot.trn_boot import _ntff_profile_via_ctypes
        mod.set_axon_ntff_profile_hook(
            _ntff_profile_via_ctypes("/opt/axon/libaxon_pjrt.so"))
    except Exception as e:
        print(f"ntff hook setup failed: {e}")


def _build(flags):
    nc = bacc.Bacc("TRN2", target_bir_lowering=False, debug=False,
                   num_devices=NCORES)

    dx0 = nc.dram_tensor("x0", [L, D], F32, kind="ExternalInput").ap()
    dwqk = nc.dram_tensor("wqk", [NL, KC, 128, 384], BF16, kind="ExternalInput").ap()
    dwv = nc.dram_tensor("wv", [NL, KC, 128, 256], BF16, kind="ExternalInput").ap()
    dwo = nc.dram_tensor("wo", [NL, 2, 128, D], BF16, kind="ExternalInput").ap()
    dw1 = nc.dram_tensor("w1", [NL, KC, 128, FFP], BF16, kind="ExternalInput").ap()
    dw2 = nc.dram_tensor("w2", [NL, KC, 128, D], BF16, kind="ExternalInput").ap()
    demb = nc.dram_tensor("embT", [KC, 128, VP], BF16, kind="ExternalInput").ap()
    dqkvb = nc.dram_tensor("qkvb", [128, NL * 4], F32, kind="ExternalInput").ap()
    df1b = nc.dram_tensor("f1b", [128, NL * 6], F32, kind="ExternalInput").ap()
    dmsk = nc.dram_tensor("msk01", [128, 128], BF16, kind="ExternalInput").ap()
    dob = dfb2 = dlgb = None
    if flags["ob"]:
        dob = nc.dram_tensor("ob", [NL, 1, D], F32, kind="ExternalInput").ap()
    if flags["fb2"]:
        dfb2 = nc.dram_tensor("fb2", [NL, 1, D], F32, kind="ExternalInput").ap()
    if flags["lgb"]:
        dlgb = nc.dram_tensor("lgb", [1, VP], F32, kind="ExternalInput").ap()
    dlog = nc.dram_tensor("logits", [L, VP], F32, kind="ExternalOutput").ap()

    groups = [[0, 1, 2, 3], [4, 5, 6, 7]]

    from contextlib import ExitStack
    with tile.TileContext(nc) as tc, ExitStack() as es:
        cst = es.enter_context(tc.tile_pool(name="cst", bufs=1))
        ph = es.enter_context(tc.tile_pool(name="ph", bufs=1))
        pxT = es.enter_context(tc.tile_pool(name="pxT", bufs=2))
        px2T = es.enter_context(tc.tile_pool(name="px2T", bufs=1))
        pqk = es.enter_context(tc.tile_pool(name="pqk", bufs=1))
        pv = es.enter_context(tc.tile_pool(name="pv", bufs=1))
        pawT = es.enter_context(tc.tile_pool(name="pawT", bufs=4))
        pattn = es.enter_context(tc.tile_pool(name="pattn", bufs=1))
        psml = es.enter_context(tc.tile_pool(name="psml", bufs=3))
        pscr = es.enter_context(tc.tile_pool(name="pscr", bufs=2))
        pff = es.enter_context(tc.tile_pool(name="pff", bufs=2))
        pcast = es.enter_context(tc.tile_pool(name="pcast", bufs=3))
        pcin = es.enter_context(tc.tile_pool(name="pcin", bufs=3))
        pw = es.enter_context(tc.tile_pool(name="pw", bufs=1))
        pemb = es.enter_context(tc.tile_pool(name="pemb", bufs=8))
        pps = es.enter_context(tc.tile_pool(name="pps", bufs=6, space="PSUM"))
        pav = es.enter_context(tc.tile_pool(name="pav", bufs=2, space="PSUM"))
        pdram = es.enter_context(tc.tile_pool(name="pdram", bufs=4,
                                              space="DRAM"))

        dma = nc.sync.dma_start

        # ---- constants ----
        ident = cst.tile([128, 128], BF16, tag="ident")
        make_identity(nc, ident[:])
        msk = cst.tile([128, 128], BF16, tag="msk")
        dma(out=msk[:], in_=dmsk[:])
        epst = cst.tile([128, 1], F32, tag="epst")
        nc.vector.memset(epst[:], EPS)
        ones1 = cst.tile([128, 1], F32, tag="ones1")
        nc.vector.memset(ones1[:], 1.0)
        ones64 = cst.tile([1, 64], BF16, tag="ones64")
        nc.scalar.copy(out=ones64[:],
                       in_=ones1[0:1, 0:1].to_broadcast([1, 64]))
        qkvb = cst.tile([128, NL * 4], F32, tag="qkvb")
        dma(out=qkvb[:], in_=dqkvb[:])
        f1b = cst.tile([128, NL * 6], F32, tag="f1b")
        dma(out=f1b[:], in_=df1b[:])
        ob_sb = fb2_sb = lgb_sb = None
        if flags["ob"]:
            ob_sb = cst.tile([128, NL * D], F32, tag="ob")
            for i in range(NL):
                dma(out=ob_sb[:, i * D:(i + 1) * D],
                    in_=dob[i].to_broadcast([128, D]))
        if flags["fb2"]:
            fb2_sb = cst.tile([128, NL * D], F32, tag="fb2")
            for i in range(NL):
                dma(out=fb2_sb[:, i * D:(i + 1) * D],
                    in_=dfb2[i].to_broadcast([128, D]))
        if flags["lgb"]:
            lgb_sb = cst.tile([128, VP], F32, tag="lgb")
            dma(out=lgb_sb[:], in_=dlgb.to_broadcast([128, VP]))

        # ---- residual stream ----
        h = ph.tile([128, TC * D], F32, tag="h")
        for t in range(TC):
            dma(out=h[:, t * D:(t + 1) * D],
                in_=dx0[t * 128:(t + 1) * 128, :])

        def ln_pair(tcs, xT, off_fn, ar=None, bias_col=None):
            """LN of token-chunks tcs into transposed slices of xT
            ([:, kc, off_fn(j) : +128] after a (p,(k t)) split). If
            ar=(dram_tile, row0): fuse h += ar into the sum reduce."""
            n = len(tcs)
            st = psml.tile([128, 6 * n], F32, tag="st", name=_nm("st"))
            S0, SQ, MU, EX, RS, NM = (0, n, 2 * n, 3 * n, 4 * n, 5 * n)
            for j, t in enumerate(tcs):
                hs = h[:, t * D:(t + 1) * D]
                if bias_col is not None:
                    nc.vector.scalar_tensor_tensor(
                        out=hs, in0=hs, scalar=1.0, in1=bias_col,
                        op0=ALU.mult, op1=ALU.add)
                if ar is not None:
                    ard, row0 = ar
                    ci = pcin.tile([128, D], BF16, tag="ci", name=_nm("ci"))
                    dma(out=ci[:],
                        in_=ard[row0 + j * 128:row0 + (j + 1) * 128, :])
                    nc.vector.scalar_tensor_tensor(
                        out=hs, in0=hs, scalar=1.0, in1=ci[:],
                        op0=ALU.mult, op1=ALU.add,
                        accum_out=st[:, S0 + j:S0 + j + 1])
                else:
                    nc.vector.tensor_reduce(out=st[:, S0 + j:S0 + j + 1],
                                            in_=hs,
                                            axis=mybir.AxisListType.X,
                                            op=ALU.add)
                scr = pscr.tile([128, D], F32, tag="scr", name=_nm("scr"))
                nc.vector.scalar_tensor_tensor(
                    out=scr[:], in0=hs, scalar=1.0, in1=hs,
                    op0=ALU.mult, op1=ALU.mult,
                    accum_out=st[:, SQ + j:SQ + j + 1])
            mu = st[:, MU:MU + n]
            ex2 = st[:, EX:EX + n]
            nc.scalar.mul(out=mu, in_=st[:, S0:S0 + n], mul=1.0 / D)
            nc.scalar.mul(out=ex2, in_=st[:, SQ:SQ + n], mul=1.0 / D)
            musq = st[:, S0:S0 + n]
            nc.scalar.square(out=musq, in_=mu)
            var = st[:, SQ:SQ + n]
            nc.vector.scalar_tensor_tensor(out=var, in0=ex2, scalar=1.0,
                                           in1=musq, op0=ALU.mult,
                                           op1=ALU.subtract)
            sd = st[:, EX:EX + n]
            nc.scalar.activation(out=sd, in_=var, func=AF.Sqrt,
                                 bias=epst[:, 0:1])
            rstd = st[:, RS:RS + n]
            nc.vector.reciprocal(out=rstd, in_=sd)
            nmr = st[:, NM:NM + n]
            nc.vector.scalar_tensor_tensor(out=nmr, in0=mu, scalar=-1.0,
                                           in1=rstd, op0=ALU.mult,
                                           op1=ALU.mult)
            for j, t in enumerate(tcs):
                z = pscr.tile([128, D], BF16, tag="zscr", name=_nm("z"))
                nc.scalar.activation(out=z[:], in_=h[:, t * D:(t + 1) * D],
                                     func=AF.Identity,
                                     bias=nmr[:, j:j + 1],
                                     scale=rstd[:, j:j + 1])
                for g in range(2):
                    ptr = pps.tile([128, 512], BF16, tag="ps",
                                   name=_nm("ptr"))
                    nk = 4 if g == 0 else 2
                    for q in range(nk):
                        kc = g * 4 + q
                        nc.tensor.transpose(ptr[:, q * 128:(q + 1) * 128],
                                            z[:, kc * 128:(kc + 1) * 128],
                                            ident[:])
                    xr = xT[:].rearrange("p (k t) -> p k t", k=KC)
                    o0 = off_fn(j)
                    nc.scalar.copy(
                        out=xr[:, g * 4:g * 4 + nk, o0:o0 + 128],
                        in_=ptr[:, 0:nk * 128].rearrange(
                            "p (k t) -> p k t", k=nk))

        # initial LN1 (layer 0)
        xT = pxT.tile([128, KC * L], BF16, tag="xT", name="xT_0")
        for tp_ in range(4):
            ln_pair([2 * tp_, 2 * tp_ + 1], xT,
                    lambda j, tp_=tp_: (2 * tp_ + j) * 128)

        MOFF = [0, 128, 192, 320]
        MSZ = [128, 64, 128, 64]

        for i in range(NL):
            wqk = pw.tile([128, KC * 384], BF16, tag="wqk", name=f"wqk_{i}")
            for kc in range(KC):
                dma(out=wqk[:, kc * 384:(kc + 1) * 384], in_=dwqk[i, kc])
            wv = pw.tile([128, KC * 256], BF16, tag="wv", name=f"wv_{i}")
            for kc in range(KC):
                dma(out=wv[:, kc * 256:(kc + 1) * 256], in_=dwv[i, kc])
            wo = pw.tile([128, 2 * D], BF16, tag="wo", name=f"wo_{i}")
            for kc in range(2):
                dma(out=wo[:, kc * D:(kc + 1) * D], in_=dwo[i, kc])
            w1 = pw.tile([128, KC * FFP], BF16, tag="w1", name=f"w1_{i}")
            for kc in range(KC):
                dma(out=w1[:, kc * FFP:(kc + 1) * FFP], in_=dw1[i, kc])
            w2 = pw.tile([128, KC * D], BF16, tag="w2", name=f"w2_{i}")
            for kc in range(KC):
                dma(out=w2[:, kc * D:(kc + 1) * D], in_=dw2[i, kc])

            # --- q/k/v projections ---
            qk = pqk.tile([128, 4 * L], BF16, tag="qk", name=f"qk_{i}")
            for m in range(4):
                msz = MSZ[m]
                for nh in range(2):
                    pmm = pps.tile([msz, 512], F32, tag="ps", name=_nm("pmm"))
                    for kc in range(KC):
                        nc.tensor.matmul(
                            pmm[:],
                            wqk[:, kc * 384 + MOFF[m]:
                                kc * 384 + MOFF[m] + msz],
                            xT[:, kc * L + nh * 512:kc * L + (nh + 1) * 512],
                            start=(kc == 0), stop=(kc == KC - 1))
                    nc.scalar.activation(
                        out=qk[0:msz, m * L + nh * 512:m * L + (nh + 1) * 512],
                        in_=pmm[:], func=AF.Identity,
                        bias=qkvb[0:msz, i * 4 + m:i * 4 + m + 1])
            v = pv.tile([128, TC * 195], BF16, tag="v", name=f"v_{i}")
            for t in range(TC):
                for hh in range(HP):
                    nc.scalar.copy(out=v[:, t * 195 + hh * 65 + 64:
                                         t * 195 + hh * 65 + 65],
                                   in_=ones1[:])
                pmm = pps.tile([128, 256], F32, tag="ps", name=_nm("pv"))
                for kc in range(KC):
                    nc.tensor.matmul(
                        pmm[:],
                        xT[:, kc * L + t * 128:kc * L + (t + 1) * 128],
                        wv[:, kc * 256:(kc + 1) * 256],
                        start=(kc == 0), stop=(kc == KC - 1))
                for hh in range(HP):
                    nc.scalar.copy(
                        out=v[:, t * 195 + hh * 65:t * 195 + hh * 65 + 64],
                        in_=pmm[:, hh * 64:(hh + 1) * 64])

            def qsl(hh, c0, c1):
                m, p0 = [(0, 0), (0, 64), (1, 0)][hh]
                return qk[p0:p0 + 64, m * L + c0:m * L + c1]

            def ksl(hh, c0, c1):
                m, p0 = [(2, 0), (2, 64), (3, 0)][hh]
                return qk[p0:p0 + 64, m * L + c0:m * L + c1]

            # --- attention (k-major, causal-skip) ---
            attnA = pattn.tile([128, L], BF16, tag="attnA", name=f"atA_{i}")
            attnB = pattn.tile([64, L], BF16, tag="attnB", name=f"atB_{i}")
            for hh in range(HP):
                for qc in range(2):
                    nkj = 4 * qc + 4
                    pavt = pav.tile([65, 512], F32, tag="av", name=_nm("pav"))
                    for kj in range(nkj):
                        off = kj - 4 * qc
                        c0 = max(0, off) * 128
                        pst = pps.tile([128, 512], F32, tag="ps",
                                       name=_nm("pst"))
                        nc.tensor.matmul(
                            pst[:, c0:512],
                            ksl(hh, kj * 128, (kj + 1) * 128),
                            qsl(hh, qc * 512 + c0, (qc + 1) * 512),
                            start=True, stop=True)
                        awT = pawT.tile([128, 512], BF16, tag="awT",
                                        name=_nm("awT"))
                        nc.scalar.activation(out=awT[:, c0:512],
                                             in_=pst[:, c0:512], func=AF.Exp)
                        if off >= 0:
                            if c0 > 0:
                                nc.vector.memset(awT[:, 0:c0], 0.0)
                            nc.vector.scalar_tensor_tensor(
                                out=awT[:, c0:c0 + 128],
                                in0=awT[:, c0:c0 + 128], scalar=1.0,
                                in1=msk[:], op0=ALU.mult, op1=ALU.mult)
                        nc.tensor.matmul(
                            pavt[:],
                            v[:, kj * 195 + hh * 65:kj * 195 + (hh + 1) * 65],
                            awT[:],
                            start=(kj == 0), stop=(kj == nkj - 1))
                    invr = psml.tile([1, 512], BF16, tag="invr",
                                     name=_nm("invr"))
                    with nc.allow_low_precision(reason="softmax inv bf16"):
                        nc.vector.reciprocal(out=invr[:], in_=pavt[64:65, :])
                    pib = pps.tile([64, 512], F32, tag="ps", name=_nm("pib"))
                    nc.tensor.matmul(pib[:], ones64[:], invr[:],
                                     start=True, stop=True)
                    ibs = psml.tile([64, 512], BF16, tag="ibs",
                                    name=_nm("ibs"))
                    nc.scalar.copy(out=ibs[:], in_=pib[:])
                    dst = attnA[hh * 64:(hh + 1) * 64,
                                qc * 512:(qc + 1) * 512] if hh < 2 else \
                        attnB[:, qc * 512:(qc + 1) * 512]
                    nc.vector.scalar_tensor_tensor(
                        out=dst, in0=pavt[0:64, :], scalar=1.0, in1=ibs[:],
                        op0=ALU.mult, op1=ALU.mult)

            # --- o_proj partials + chunked AllReduce (4 x 256 tokens) ---
            obnc = [pdram.tile([256, D], BF16, tag="obnc",
                               name=f"obnc_{i}_{k}") for k in range(4)]
            ored = [pdram.tile([256, D], BF16, tag="ored",
                               name=f"ored_{i}_{k}") for k in range(4)]
            for t in range(TC):
                oc = pcast.tile([128, D], BF16, tag="oc", name=_nm("oc"))
                for nn_ in range(2):
                    n0, n1 = nn_ * 512, min(D, (nn_ + 1) * 512)
                    pmo = pps.tile([128, n1 - n0], F32, tag="ps",
                                   name=_nm("pmo"))
                    nc.tensor.matmul(pmo[:],
                                     attnA[:, t * 128:(t + 1) * 128],
                                     wo[:, n0:n1], start=True, stop=False)
                    nc.tensor.matmul(pmo[:],
                                     attnB[:, t * 128:(t + 1) * 128],
                                     wo[0:64, D + n0:D + n1],
                                     start=False, stop=True)
                    nc.vector.tensor_scalar_add(out=oc[:, n0:n1],
                                                in0=pmo[:], scalar1=0.0)
                dma(out=obnc[t // 2][(t % 2) * 128:(t % 2) * 128 + 128, :],
                    in_=oc[:])
                if t % 2 == 1:
                    nc.gpsimd.collective_compute(
                        "AllReduce", ALU.add, replica_groups=groups,
                        ins=[obnc[t // 2].opt()], outs=[ored[t // 2].opt()])

            # --- LN2 + FFN per 256-token chunk, chunked ff AllReduce ---
            fbnc = [pdram.tile([256, D], BF16, tag="fbnc",
                               name=f"fbnc_{i}_{k}") for k in range(4)]
            fred = [pdram.tile([256, D], BF16, tag="fred",
                               name=f"fred_{i}_{k}") for k in range(4)]
            bias_col = ob_sb[:, i * D:(i + 1) * D] if flags["ob"] else None
            for tp_ in range(4):
                x2T = px2T.tile([128, KC * 256], BF16, tag="x2T",
                                name=_nm("x2T"))
                ln_pair([2 * tp_, 2 * tp_ + 1], x2T,
                        lambda j: j * 128, ar=(ored[tp_], 0),
                        bias_col=bias_col)
                ff = pff.tile([128, KC * 256], BF16, tag="ff", name=_nm("ff"))
                for mc in range(KC):
                    pmf = pps.tile([128, 256], F32, tag="ps", name=_nm("pmf"))
                    for kc in range(KC):
                        nc.tensor.matmul(
                            pmf[:],
                            w1[:, kc * FFP + mc * 128:
                               kc * FFP + (mc + 1) * 128],
                            x2T[:, kc * 256:(kc + 1) * 256],
                            start=(kc == 0), stop=(kc == KC - 1))
                    nc.scalar.activation(
                        out=ff[:, mc * 256:(mc + 1) * 256],
                        in_=pmf[:], func=AF.Gelu,
                        bias=f1b[:, i * 6 + mc:i * 6 + mc + 1])
                for sub in range(2):
                    t = 2 * tp_ + sub
                    fc = pcast.tile([128, D], BF16, tag="fc", name=_nm("fc"))
                    for nn_ in range(2):
                        n0, n1 = nn_ * 512, min(D, (nn_ + 1) * 512)
                        pmf2 = pps.tile([128, n1 - n0], F32, tag="ps",
                                        name=_nm("pmf2"))
                        for kc in range(KC):
                            nc.tensor.matmul(
                                pmf2[:],
                                ff[:, kc * 256 + sub * 128:
                                   kc * 256 + sub * 128 + 128],
                                w2[:, kc * D + n0:kc * D + n1],
                                start=(kc == 0), stop=(kc == KC - 1))
                        nc.vector.tensor_scalar_add(out=fc[:, n0:n1],
                                                    in0=pmf2[:], scalar1=0.0)
                    dma(out=fbnc[tp_][sub * 128:sub * 128 + 128, :],
                        in_=fc[:])
                nc.gpsimd.collective_compute(
                    "AllReduce", ALU.add, replica_groups=groups,
                    ins=[fbnc[tp_].opt()], outs=[fred[tp_].opt()])

            # --- consume ff-AR: h update fused with next LN ---
            nxT = pxT.tile([128, KC * L], BF16, tag="xT", name=f"xT_{i + 1}")
            bias_col2 = fb2_sb[:, i * D:(i + 1) * D] if flags["fb2"] else None
            for tp_ in range(4):
                ln_pair([2 * tp_, 2 * tp_ + 1], nxT,
                        lambda j, tp_=tp_: (2 * tp_ + j) * 128,
                        ar=(fred[tp_], 0), bias_col=bias_col2)
            xT = nxT

        # ======= logits (xT now holds lnf(h) transposed) =======
        zT = xT
        nvc = VP // 512
        for vc in range(nvc):
            v0, v1 = vc * 512, (vc + 1) * 512
            et = [pemb.tile([128, 512], BF16, tag="emb",
                            name=f"emb_{vc}_{k}") for k in range(KC)]
            for kc in range(KC):
                dma(out=et[kc][:], in_=demb[kc, :, v0:v1])
            for t in range(TC):
                pml = pps.tile([128, 512], F32, tag="ps",
                               name=f"pml_{vc}_{t}")
                for kc in range(KC):
                    nc.tensor.matmul(
                        pml[:],
                        zT[:, kc * L + t * 128:kc * L + (t + 1) * 128],
                        et[kc][:],
                        start=(kc == 0), stop=(kc == KC - 1))
                lg = pscr.tile([128, D], F32, tag="scr",
                               name=f"lgout_{vc}_{t}")[:, 0:512]
                if flags["lgb"]:
                    nc.vector.scalar_tensor_tensor(
                        out=lg, in0=pml[:], scalar=1.0,
                        in1=lgb_sb[:, v0:v1], op0=ALU.mult, op1=ALU.add)
                elif t % 2 == 0:
                    nc.vector.tensor_scalar_add(out=lg, in0=pml[:],
                                                scalar1=0.0)
                else:
                    nc.scalar.copy(out=lg, in_=pml[:])
                dma(out=dlog[t * 128:(t + 1) * 128, v0:v1], in_=lg)

    nc.compile()
    return nc


def _prep_inputs(tokens, timelike_mask, embed, pos_emb, wq, wk, wv, wo,
                 ln1_g, ln1_b, ln2_g, ln2_b, ff_w1, ff_b1, ff_w2, ff_b2,
                 lnf_g, lnf_b):
    import ml_dtypes
    bf = ml_dtypes.bfloat16
    f32 = np.float32
    tokens = np.asarray(tokens)
    scale = float(np.sqrt(DH))
    flags = {
        "ob": bool(np.any(ln1_b)),
        "fb2": bool(np.any(ff_b2) or np.any(ln2_b)),
        "lgb": bool(np.any(lnf_b)),
    }

    x0 = (np.asarray(embed)[tokens] +
          np.asarray(pos_emb)[None, :L]).astype(f32)

    i_idx = np.arange(128)[:, None]
    j_idx = np.arange(128)[None, :]
    msk01 = (j_idx >= i_idx).astype(f32).astype(bf)

    per_rank = []
    for t in range(TP):
        fs, fe = t * FP, (t + 1) * FP
        ffs, ffe = t * FFP, (t + 1) * FFP
        vs = t * VS
        ve = min(VOCAB, (t + 1) * VS)

        wqk_r = np.zeros((NL, KC, 128, 384), f32)
        wv_r = np.zeros((NL, KC, 128, 256), f32)
        wo_r = np.zeros((NL, 2, 128, D), f32)
        w1_r = np.zeros((NL, KC, 128, FFP), f32)
        w2_r = np.zeros((NL, KC, 128, D), f32)
        qkvb_r = np.zeros((128, NL * 4), f32)
        f1b_r = np.zeros((128, NL * 6), f32)
        ob_r = np.zeros((NL, 1, D), f32)
        fb2_r = np.zeros((NL, 1, D), f32)

        for i in range(NL):
            s_lor = (1.0 - 2.0 * ALPHA *
                     np.asarray(timelike_mask)[i].astype(f32)) / scale
            wq_g = (np.asarray(wq)[i] * s_lor[:, None]) * \
                np.asarray(ln1_g)[i][None, :]
            wk_g = np.asarray(wk)[i] * np.asarray(ln1_g)[i][None, :]
            wv_g = np.asarray(wv)[i] * np.asarray(ln1_g)[i][None, :]
            qb = wq_g[fs:fe] @ np.asarray(ln1_b)[i]
            kb = wk_g[fs:fe] @ np.asarray(ln1_b)[i]
            qkT = wq_g[fs:fe].T
            kkT = wk_g[fs:fe].T
            pack = np.concatenate([qkT, kkT], 1)
            wqk_r[i] = pack.reshape(KC, 128, 384)
            wv_r[i, :, :, 0:FP] = wv_g[fs:fe].T.reshape(KC, 128, FP)
            woT = np.asarray(wo)[i][:, fs:fe].T
            wo_r[i, 0] = woT[0:128]
            wo_r[i, 1, 0:64] = woT[128:192]
            w1_g = np.asarray(ff_w1)[i][ffs:ffe] * \
                np.asarray(ln2_g)[i][None, :]
            w1_r[i] = w1_g.T.reshape(KC, 128, FFP)
            w2_r[i] = np.asarray(ff_w2)[i][:, ffs:ffe].T.reshape(KC, 128, D)
            qkvb_r[:, i * 4 + 0] = qb[0:128]
            qkvb_r[0:64, i * 4 + 1] = qb[128:192]
            qkvb_r[:, i * 4 + 2] = kb[0:128]
            qkvb_r[0:64, i * 4 + 3] = kb[128:192]
            b1 = w1_g @ np.asarray(ln2_b)[i] + np.asarray(ff_b1)[i][ffs:ffe]
            f1b_r[:, i * 6:(i + 1) * 6] = b1.reshape(KC, 128).T
            vb = wv_g[fs:fe] @ np.asarray(ln1_b)[i]
            ob_r[i, 0] = np.asarray(wo)[i][:, fs:fe] @ vb
            fb2_r[i, 0] = np.asarray(ff_b2)[i]

        embT_r = np.zeros((KC, 128, VP), f32)
        esl = (np.asarray(embed)[vs:ve] * np.asarray(lnf_g)[None, :]).T
        embT_r[:, :, 0:ve - vs] = esl.reshape(KC, 128, ve - vs)
        lgb_r = np.zeros((1, VP), f32)
        lgb_r[0, 0:ve - vs] = np.asarray(embed)[vs:ve] @ np.asarray(lnf_b)

        per_rank.append(dict(
            wqk=wqk_r.astype(bf), wv=wv_r.astype(bf), wo=wo_r.astype(bf),
            w1=w1_r.astype(bf), w2=w2_r.astype(bf),
            qkvb=qkvb_r, f1b=f1b_r, embT=embT_r.astype(bf),
            ob=ob_r, fb2=fb2_r, lgb=lgb_r, msk01=msk01))

    in_maps = []
    for c in range(NCORES):
        g, t = c // TP, c % TP
        m = dict(per_rank[t])
        m["x0"] = np.ascontiguousarray(x0[g])
        if not flags["ob"]:
            m.pop("ob")
        if not flags["fb2"]:
            m.pop("fb2")
        if not flags["lgb"]:
            m.pop("lgb")
        in_maps.append(m)
    return in_maps, flags


def kernel(**inputs):
    in_maps, flags = _prep_inputs(**inputs)
    key = tuple(sorted(flags.items()))
    if key not in _cached:
        _cached[key] = _build(flags)
    nc = _cached[key]
    global LAST_EXEC_NS, LAST_TRACE_DIR
    if TRACE:
        _ensure_ntff_hook()
        import tempfile
        tdir = tempfile.mkdtemp(prefix="lorentz_trace_")
        res = run_bass_kernel_spmd(nc, in_maps, core_ids=list(range(NCORES)),
                                   trace=True, tmpdir=tdir)
        LAST_EXEC_NS = res.exec_time_ns
        LAST_TRACE_DIR = tdir
    else:
        res = run_bass_kernel_spmd(nc, in_maps, core_ids=list(range(NCORES)))
    out = np.zeros((B, L, VOCAB), np.float32)
    for c in range(NCORES):
        g, t = c // TP, c % TP
        vs = t * VS
        ve = min(VOCAB, (t + 1) * VS)
        out[g, :, vs:ve] = res.results[c]["logits"][:, 0:ve - vs]
    return out
